# revision 9
# baseline (speedup 1.0000x reference)
"""Trainium2 Bass kernel for nn_AttentionModule (conv3x3 -> BN -> LeakyReLU ->
spatial attention -> residual -> LN -> LeakyReLU).

Key simplification: the reference computes softmax(k, axis=N).sum(axis=N) which
is identically 1 (softmax sums to one over its own axis), so s1 = s2 = 1,
p1 = q, att = v. The q/k convs and both softmaxes never affect the output.
The module reduces to:
    x = leaky(BN(conv3x3(inputs)))          # batch-stat BN, eps=1e-3
    y = x + conv1x1(x, wv) + bv             # folded: conv1x1(x, wv + I) + bv
    out = leaky(LN(y))                      # per-sample LN, eps=1e-3
(conv bias cbl_b cancels inside train-mode BN; wq/bq/wk/bk are dead.)

Sharding: pure data-parallel over batch (2 images per core on 8 cores) with a
512-float AllReduce of the BN statistics.

Device layout is channel-major ([C_chunk=128 partitions, pixels free]); the
host pre-transposes/pads inputs and transposes the output back, so all device
DMA is contiguous.
"""

import numpy as np

import concourse.bacc as bacc
import concourse.tile as tile
from concourse import mybir
from concourse.bass_utils import run_bass_kernel_spmd

B, H, W, CIN, C = 16, 64, 64, 128, 256
NCORES = 8
BL = B // NCORES            # images per core
HP, WP = H + 2, W + 2       # padded spatial dims
PIX = BL * H * W            # pixels per core (8192)
NPIXG = B * H * W           # global pixel count for BN stats (65536)
EPS = 1e-3
F32 = mybir.dt.float32
AF = mybir.ActivationFunctionType
OP = mybir.AluOpType

NGROUP = PIX // 512         # 16 PSUM-sized pixel groups per core
ALPHA = 0.3                 # LeakyReLU slope

_CACHE = {}
LAST_RESULT = None


def _build(fast_ln: bool):
    nc = bacc.Bacc("TRN2", num_devices=NCORES)

    xin = nc.dram_tensor("xin", [CIN, BL * HP * WP], F32, kind="ExternalInput")
    cw = nc.dram_tensor("cw", [CIN, 9 * C], F32, kind="ExternalInput")
    wv = nc.dram_tensor("wv", [C, C], F32, kind="ExternalInput")
    bnp = nc.dram_tensor("bnp", [C, 3], F32, kind="ExternalInput")  # gamma, beta, bv
    if not fast_ln:
        lng = nc.dram_tensor("lng", [C, H * W], F32, kind="ExternalInput")
        lnb = nc.dram_tensor("lnb", [C, H * W], F32, kind="ExternalInput")
    yout = nc.dram_tensor("yout", [C, PIX], F32, kind="ExternalOutput")
    cc_in = nc.dram_tensor("cc_in", [C, 2], F32)
    cc_out = nc.dram_tensor("cc_out", [C, 2], F32, addr_space="Shared")

    with tile.TileContext(nc) as tc:
        with tc.tile_pool(name="wpool", bufs=1) as wpool, \
             tc.tile_pool(name="stat", bufs=1) as stat, \
             tc.tile_pool(name="Xp", bufs=2) as Xp, \
             tc.tile_pool(name="ps", bufs=6, space="PSUM") as ps:

            # ---- weight / param loads ----
            wt = wpool.tile([CIN, 9, C], F32, tag="wt")
            nc.sync.dma_start(out=wt[:], in_=cw.ap()[:].rearrange("k (t c) -> k t c", t=9))
            wvt = wpool.tile([128, 2, C], F32, tag="wvt")
            for kc in range(2):
                nc.sync.dma_start(out=wvt[:, kc, :], in_=wv.ap()[kc * 128:(kc + 1) * 128, :])
            bnpt = stat.tile([128, 2, 3], F32, tag="bnpt")
            for ch in range(2):
                nc.sync.dma_start(out=bnpt[:, ch, :], in_=bnp.ap()[ch * 128:(ch + 1) * 128, :])

            X = [Xp.tile([128, PIX], F32, tag="X", name=f"X{i}") for i in range(2)]
            bnstat = stat.tile([128, 2, NGROUP, 6], F32, tag="bnstat")

            with tc.tile_pool(name="xtp", bufs=1) as xtp:
                # ---- padded input load (4 pieces: per image halves) ----
                xt = xtp.tile([CIN, BL, HP, WP], F32, tag="xt")
                xv = xin.ap()[:].rearrange("k (b h w) -> k b h w", b=BL, h=HP)
                for b in range(BL):
                    nc.sync.dma_start(out=xt[:, b, 0:34, :], in_=xv[:, b, 0:34, :])
                    nc.sync.dma_start(out=xt[:, b, 34:HP, :], in_=xv[:, b, 34:HP, :])

                # ---- conv3x3 as 9 accumulated matmuls ----
                for ch in range(2):
                    for q in range(4):            # quarters of 4 groups
                        accs = [ps.tile([128, 512], F32, tag="ps", name=f"acc_{ch}_{q}_{gi}") for gi in range(4)]
                        b = q // 2
                        for tap in range(9):
                            dy, dx = tap // 3, tap % 3
                            lhsT = wt[:, tap, ch * 128:(ch + 1) * 128]
                            for gi in range(4):
                                r0 = (q % 2) * 32 + gi * 8
                                rhs = xt[:, b, r0 + dy:r0 + dy + 8, dx:dx + W]
                                nc.tensor.matmul(accs[gi], lhsT, rhs,
                                                 start=(tap == 0), stop=(tap == 8))
                        for gi in range(4):
                            g = q * 4 + gi
                            seg = X[ch][:, g * 512:(g + 1) * 512]
                            nc.scalar.activation(out=seg, in_=accs[gi], func=AF.Copy)
                            nc.vector.bn_stats(out=bnstat[:, ch, g, :], in_=seg)

            # ---- BN stats -> global sums via AllReduce ----
            mv = stat.tile([128, 2, 2], F32, tag="mv")
            sums = stat.tile([128, 2, 2], F32, tag="sums")
            for ch in range(2):
                nc.vector.bn_aggr(out=mv[:, ch, :], in_=bnstat[:, ch, :, :])
                mean, var = mv[:, ch, 0:1], mv[:, ch, 1:2]
                nc.vector.tensor_scalar_mul(sums[:, ch, 0:1], mean, float(PIX))
                # sumsq = (mean*mean + var) * PIX
                nc.vector.tensor_scalar(sums[:, ch, 1:2], mean, mean, var, OP.mult, OP.add)
                nc.vector.tensor_scalar_mul(sums[:, ch, 1:2], sums[:, ch, 1:2], float(PIX))
                nc.sync.dma_start(out=cc_in.ap()[ch * 128:(ch + 1) * 128, :], in_=sums[:, ch, :])
            nc.gpsimd.collective_compute(
                "AllReduce", OP.add, replica_groups=[list(range(NCORES))],
                ins=[cc_in.ap()[:]], outs=[cc_out.ap()[:]])
            gsum = stat.tile([128, 2, 2], F32, tag="gsum")
            for ch in range(2):
                nc.sync.dma_start(out=gsum[:, ch, :], in_=cc_out.ap()[ch * 128:(ch + 1) * 128, :])

            # ---- BN scale/bias: s = gamma*rsqrt(var+eps), bb = beta - mu*s ----
            sbn = stat.tile([128, 2], F32, tag="sbn")
            bbn = stat.tile([128, 2], F32, tag="bbn")
            tmp = stat.tile([128, 2, 2], F32, tag="tmpbn")
            eps128 = stat.tile([128, 1], F32, tag="eps128")
            nc.vector.memset(eps128[:], EPS)
            for ch in range(2):
                mu, ex2 = tmp[:, ch, 0:1], tmp[:, ch, 1:2]
                nc.vector.tensor_scalar_mul(mu, gsum[:, ch, 0:1], 1.0 / NPIXG)
                nc.vector.tensor_scalar_mul(ex2, gsum[:, ch, 1:2], 1.0 / NPIXG)
                var = sbn[:, ch:ch + 1]
                nc.vector.tensor_scalar(var, mu, mu, None, OP.mult)
                nc.vector.tensor_sub(var, ex2, var)
                nc.scalar.activation(out=var, in_=var, func=AF.Sqrt, bias=eps128[:])
                nc.vector.reciprocal(out=var, in_=var)
                nc.vector.tensor_mul(var, var, bnpt[:, ch, 0:1])   # s = rstd * gamma
                nc.vector.tensor_mul(mu, mu, var)                  # mu*s
                nc.vector.tensor_sub(bbn[:, ch:ch + 1], bnpt[:, ch, 1:2], mu)

            # ---- phase 2: BN-apply+leaky -> conv1x1(+residual) -> LN stats ----
            with tc.tile_pool(name="yp", bufs=2) as yp, \
                 tc.tile_pool(name="blk", bufs=4) as blk:
                Y = [yp.tile([128, PIX], F32, tag="y", name=f"Y{i}") for i in range(2)]
                lnstat = stat.tile([128, 2, BL, 8, 6], F32, tag="lnstat")
                for bi in range(4):              # blocks of 2048 pixels
                    xbs = [blk.tile([128, 2048], F32, tag="xb", name=f"xb_{bi}_{kc}") for kc in range(2)]
                    for kc in range(2):
                        nc.scalar.activation(
                            out=xbs[kc][:], in_=X[kc][:, bi * 2048:(bi + 1) * 2048],
                            func=AF.Prelu, bias=bbn[:, kc:kc + 1], scale=sbn[:, kc:kc + 1],
                            alpha=ALPHA)
                    for ch in range(2):
                        for sl in range(4):
                            acc = ps.tile([128, 512], F32, tag="ps")
                            for kc in range(2):
                                nc.tensor.matmul(
                                    acc, wvt[:, kc, ch * 128:(ch + 1) * 128],
                                    xbs[kc][:, sl * 512:(sl + 1) * 512],
                                    start=(kc == 0), stop=(kc == 1))
                            seg = Y[ch][:, bi * 2048 + sl * 512: bi * 2048 + (sl + 1) * 512]
                            nc.scalar.activation(out=seg, in_=acc, func=AF.Identity,
                                                 bias=bnpt[:, ch, 2:3], scale=1.0)
                            nc.vector.bn_stats(out=lnstat[:, ch, bi // 2, (bi % 2) * 4 + sl, :],
                                               in_=seg)

                # ---- LN: combine per-channel stats across 256 channels ----
                rhsT = stat.tile([128, 2, 4], F32, tag="rhsT")   # (m_b0, e2_b0, m_b1, e2_b1)
                mvb = stat.tile([128, 2], F32, tag="mvb")
                for ch in range(2):
                    for b in range(BL):
                        nc.vector.bn_aggr(out=mvb[:], in_=lnstat[:, ch, b, :, :])
                        mean, var = mvb[:, 0:1], mvb[:, 1:2]
                        nc.vector.tensor_copy(rhsT[:, ch, 2 * b:2 * b + 1], mean)
                        nc.vector.tensor_scalar(rhsT[:, ch, 2 * b + 1:2 * b + 2],
                                                mean, mean, var, OP.mult, OP.add)
                ones = stat.tile([128, 1], F32, tag="ones")
                nc.vector.memset(ones[:], 1.0)
                with tc.tile_pool(name="pss", bufs=1, space="PSUM") as pss:
                    psum4 = pss.tile([1, 4], F32, tag="ps4")
                    for ch in range(2):
                        nc.tensor.matmul(psum4[:], ones[:], rhsT[:, ch, :],
                                         start=(ch == 0), stop=(ch == 1))
                    t4 = stat.tile([1, 2, 2], F32, tag="t4")     # [b, (m, e2)]
                    nc.scalar.activation(out=t4[:],
                                         in_=psum4[:].rearrange("p (b s) -> p b s", b=2),
                                         func=AF.Copy, scale=1.0 / C)
                    eps1 = stat.tile([1, 1], F32, tag="eps1")
                    nc.vector.memset(eps1[:], EPS)
                    m2 = stat.tile([1, 2], F32, tag="m2")
                    nc.vector.tensor_mul(m2[:], t4[:, :, 0], t4[:, :, 0])
                    varb = stat.tile([1, 2], F32, tag="varb")
                    nc.vector.tensor_sub(varb[:], t4[:, :, 1], m2[:])
                    nc.scalar.activation(out=varb[:], in_=varb[:], func=AF.Sqrt, bias=eps1[:])
                    nc.vector.reciprocal(out=varb[:], in_=varb[:])   # r_b
                    bsrc = stat.tile([1, 4], F32, tag="bsrc")        # (m0, m1, r0, r1)
                    nc.vector.tensor_copy(bsrc[:, 0:2], t4[:, :, 0])
                    nc.vector.tensor_copy(bsrc[:, 2:4], varb[:])
                    ones1 = stat.tile([1, 128], F32, tag="ones1")
                    nc.vector.memset(ones1[:], 1.0)
                    psbc = pss.tile([128, 4], F32, tag="psbc")
                    nc.tensor.matmul(psbc[:], ones1[:], bsrc[:], start=True, stop=True)
                    bc = stat.tile([128, 4], F32, tag="bc")          # (m0, m1, r0, r1) bcast
                    nc.scalar.activation(out=bc[:], in_=psbc[:], func=AF.Copy)
                lnbias = stat.tile([128, 2], F32, tag="lnbias")      # -m_b * r_b
                nc.vector.tensor_mul(lnbias[:], bc[:, 0:2], bc[:, 2:4])
                nc.vector.tensor_scalar_mul(lnbias[:], lnbias[:], -1.0)

                # ---- final: out = leaky((y - m_b) * r_b [* g + beta]) ----
                if fast_ln:
                    for ch in range(2):
                        outt = Xp.tile([128, PIX], F32, tag="X")
                        for b in range(BL):
                            nc.scalar.activation(
                                out=outt[:, b * 4096:(b + 1) * 4096],
                                in_=Y[ch][:, b * 4096:(b + 1) * 4096],
                                func=AF.Prelu, bias=lnbias[:, b:b + 1],
                                scale=bc[:, 2 + b:3 + b], alpha=ALPHA)
                        nc.sync.dma_start(out=yout.ap()[ch * 128:(ch + 1) * 128, :], in_=outt[:])
                else:
                    with tc.tile_pool(name="lnp", bufs=2) as lnp:
                        for ch in range(2):
                            gam = lnp.tile([128, H * W], F32, tag="gam")
                            bet = lnp.tile([128, H * W], F32, tag="bet")
                            nc.sync.dma_start(out=gam[:], in_=lng.ap()[ch * 128:(ch + 1) * 128, :])
                            nc.sync.dma_start(out=bet[:], in_=lnb.ap()[ch * 128:(ch + 1) * 128, :])
                            outt = Xp.tile([128, PIX], F32, tag="X")
                            for b in range(BL):
                                seg = outt[:, b * 4096:(b + 1) * 4096]
                                nc.scalar.activation(
                                    out=seg, in_=Y[ch][:, b * 4096:(b + 1) * 4096],
                                    func=AF.Identity, bias=lnbias[:, b:b + 1],
                                    scale=bc[:, 2 + b:3 + b])
                                nc.vector.tensor_mul(seg, seg, gam[:])
                                nc.vector.tensor_add(seg, seg, bet[:])
                                nc.scalar.activation(out=seg, in_=seg, func=AF.Prelu,
                                                     bias=0.0, scale=1.0, alpha=ALPHA)
                            nc.sync.dma_start(out=yout.ap()[ch * 128:(ch + 1) * 128, :], in_=outt[:])

    nc.compile()
    return nc


def kernel(**inputs):
    global LAST_RESULT
    x = np.ascontiguousarray(np.asarray(inputs["inputs"], dtype=np.float32))
    cbl_w = np.asarray(inputs["cbl_w"], dtype=np.float32)
    bn_gamma = np.asarray(inputs["bn_gamma"], dtype=np.float32)
    bn_beta = np.asarray(inputs["bn_beta"], dtype=np.float32)
    wv = np.asarray(inputs["wv"], dtype=np.float32).reshape(C, C)
    bv = np.asarray(inputs["bv"], dtype=np.float32)
    ln_gamma = np.asarray(inputs["ln_gamma"], dtype=np.float32)
    ln_beta = np.asarray(inputs["ln_beta"], dtype=np.float32)

    fast_ln = bool(np.all(ln_gamma == 1.0) and np.all(ln_beta == 0.0))

    # host-side repack (free for HW time): channel-major, pre-padded input
    xp = np.zeros((NCORES, CIN, BL, HP, WP), np.float32)
    xp[:, :, :, 1:H + 1, 1:W + 1] = (
        x.reshape(NCORES, BL, H, W, CIN).transpose(0, 4, 1, 2, 3))
    xin = np.ascontiguousarray(xp.reshape(NCORES, CIN, BL * HP * WP))
    cw = np.ascontiguousarray(cbl_w.transpose(2, 0, 1, 3).reshape(CIN, 9 * C))
    wv_eff = np.ascontiguousarray(wv + np.eye(C, dtype=np.float32))
    bnp = np.ascontiguousarray(np.stack([bn_gamma, bn_beta, bv], axis=1))

    if fast_ln not in _CACHE:
        _CACHE[fast_ln] = _build(fast_ln)
    nc = _CACHE[fast_ln]

    in_maps = []
    for i in range(NCORES):
        m = {"xin": xin[i], "cw": cw, "wv": wv_eff, "bnp": bnp}
        if not fast_ln:
            m["lng"] = np.ascontiguousarray(
                ln_gamma.transpose(2, 0, 1).reshape(C, H * W))
            m["lnb"] = np.ascontiguousarray(
                ln_beta.transpose(2, 0, 1).reshape(C, H * W))
        in_maps.append(m)

    res = run_bass_kernel_spmd(nc, in_maps, core_ids=list(range(NCORES)))
    LAST_RESULT = res

    out = np.empty((B, H, W, C), np.float32)
    for i in range(NCORES):
        yc = res.results[i]["yout"].reshape(C, BL, H, W)
        out[i * BL:(i + 1) * BL] = yc.transpose(1, 2, 3, 0)
    return out


# revision 10
# speedup vs baseline: 2.5047x; 2.5047x over previous
"""Trainium2 Bass kernel for nn_AttentionModule (conv3x3 -> BN -> LeakyReLU ->
spatial attention -> residual -> LN -> LeakyReLU).

Key simplification: the reference computes softmax(k, axis=N).sum(axis=N) which
is identically 1 (softmax sums to one over its own axis), so s1 = s2 = 1,
p1 = q, att = v. The q/k convs and both softmaxes never affect the output.
The module reduces to:
    x = leaky(BN(conv3x3(inputs)))          # batch-stat BN, eps=1e-3
    y = x + conv1x1(x, wv) + bv             # folded: conv1x1(x, wv + I) + bv
    out = leaky(LN(y))                      # per-sample LN, eps=1e-3
(conv bias cbl_b cancels inside train-mode BN; wq/bq/wk/bk are dead.)

Sharding: pure data-parallel over batch (2 images per core on 8 cores) with a
per-chunk 256-float AllReduce of the BN statistics, overlapped with the other
channel-chunk's convolution.

Matmuls run in float32r (TF32-like, 1 cycle/row vs fp32's 4) — measured
~1.5e-4 relative error on the conv versus 2.3e-3 for bf16.

Device layout is channel-major ([C_chunk=128 partitions, pixels free]); the
host pre-transposes/pads inputs and transposes the output back, so all device
DMA is contiguous.
"""

import numpy as np

import concourse.bacc as bacc
import concourse.tile as tile
from concourse import mybir
from concourse.bass_utils import run_bass_kernel_spmd

B, H, W, CIN, C = 16, 64, 64, 128, 256
NCORES = 8
BL = B // NCORES            # images per core
HP, WP = H + 2, W + 2       # padded spatial dims
PIX = BL * H * W            # pixels per core (8192)
NPIXG = B * H * W           # global pixel count for BN stats (65536)
EPS = 1e-3
F32 = mybir.dt.float32
F32R = mybir.dt.float32r
AF = mybir.ActivationFunctionType
OP = mybir.AluOpType

NGROUP = PIX // 512         # 16 PSUM-sized pixel groups per core
ALPHA = 0.3                 # LeakyReLU slope

_CACHE = {}
LAST_RESULT = None


def _build(fast_ln: bool):
    nc = bacc.Bacc("TRN2", num_devices=NCORES)

    xin = nc.dram_tensor("xin", [CIN, BL * HP * WP], F32R, kind="ExternalInput")
    cw = nc.dram_tensor("cw", [CIN, 9 * C], F32R, kind="ExternalInput")
    wv = nc.dram_tensor("wv", [C, C], F32R, kind="ExternalInput")
    bnp = nc.dram_tensor("bnp", [C, 3], F32, kind="ExternalInput")  # gamma, beta, bv
    if not fast_ln:
        lng = nc.dram_tensor("lng", [C, H * W], F32, kind="ExternalInput")
        lnb = nc.dram_tensor("lnb", [C, H * W], F32, kind="ExternalInput")
    yout = nc.dram_tensor("yout", [C, PIX], F32, kind="ExternalOutput")
    cc_in = [nc.dram_tensor(f"cc_in{ch}", [128, 2], F32) for ch in range(2)]
    cc_out = [nc.dram_tensor(f"cc_out{ch}", [128, 2], F32, addr_space="Shared")
              for ch in range(2)]

    with tile.TileContext(nc) as tc:
        with tc.tile_pool(name="wpool", bufs=1) as wpool, \
             tc.tile_pool(name="stat", bufs=1) as stat, \
             tc.tile_pool(name="Xp", bufs=2) as Xp, \
             tc.tile_pool(name="ps", bufs=6, space="PSUM") as ps:

            # ---- weight / param loads ----
            wt = wpool.tile([CIN, 9, C], F32R, tag="wt")
            nc.sync.dma_start(out=wt[:], in_=cw.ap()[:].rearrange("k (t c) -> k t c", t=9))
            wvt = wpool.tile([128, 2, C], F32R, tag="wvt")
            for kc in range(2):
                nc.sync.dma_start(out=wvt[:, kc, :], in_=wv.ap()[kc * 128:(kc + 1) * 128, :])
            bnpt = stat.tile([128, 2, 3], F32, tag="bnpt")
            for ch in range(2):
                nc.sync.dma_start(out=bnpt[:, ch, :], in_=bnp.ap()[ch * 128:(ch + 1) * 128, :])

            X = [Xp.tile([128, PIX], F32, tag="X", name=f"X{i}") for i in range(2)]
            bnstat = stat.tile([128, 2, NGROUP, 6], F32, tag="bnstat")
            mv = stat.tile([128, 2, 2], F32, tag="mv")
            sums = stat.tile([128, 2, 2], F32, tag="sums")
            eps128 = stat.tile([128, 1], F32, tag="eps128")
            nc.vector.memset(eps128[:], EPS)

            with tc.tile_pool(name="xtp", bufs=1) as xtp:
                # ---- padded input load (4 pieces: per image halves) ----
                xt = xtp.tile([CIN, BL, HP, WP], F32R, tag="xt")
                xv = xin.ap()[:].rearrange("k (b h w) -> k b h w", b=BL, h=HP)
                for b in range(BL):
                    nc.sync.dma_start(out=xt[:, b, 0:34, :], in_=xv[:, b, 0:34, :])
                    nc.sync.dma_start(out=xt[:, b, 34:HP, :], in_=xv[:, b, 34:HP, :])

                # ---- conv3x3 (9 accumulated f32r matmuls) + per-chunk BN AR ----
                for ch in range(2):
                    for q in range(4):            # quarters of 4 groups
                        accs = [ps.tile([128, 512], F32, tag="ps",
                                        name=f"acc_{ch}_{q}_{gi}") for gi in range(4)]
                        b = q // 2
                        for tap in range(9):
                            dy, dx = tap // 3, tap % 3
                            lhsT = wt[:, tap, ch * 128:(ch + 1) * 128]
                            for gi in range(4):
                                r0 = (q % 2) * 32 + gi * 8
                                rhs = xt[:, b, r0 + dy:r0 + dy + 8, dx:dx + W]
                                nc.tensor.matmul(accs[gi], lhsT, rhs,
                                                 start=(tap == 0), stop=(tap == 8))
                        for gi in range(4):
                            g = q * 4 + gi
                            seg = X[ch][:, g * 512:(g + 1) * 512]
                            nc.scalar.activation(out=seg, in_=accs[gi], func=AF.Copy)
                            nc.vector.bn_stats(out=bnstat[:, ch, g, :], in_=seg)
                    # local stats -> global sums; per-chunk AllReduce so the
                    # first one overlaps the second chunk's conv.
                    nc.vector.bn_aggr(out=mv[:, ch, :], in_=bnstat[:, ch, :, :])
                    mean, var = mv[:, ch, 0:1], mv[:, ch, 1:2]
                    nc.vector.tensor_scalar_mul(sums[:, ch, 0:1], mean, float(PIX))
                    nc.vector.tensor_scalar(sums[:, ch, 1:2], mean, mean, var, OP.mult, OP.add)
                    nc.vector.tensor_scalar_mul(sums[:, ch, 1:2], sums[:, ch, 1:2], float(PIX))
                    nc.sync.dma_start(out=cc_in[ch].ap()[:], in_=sums[:, ch, :])
                    nc.gpsimd.collective_compute(
                        "AllReduce", OP.add, replica_groups=[list(range(NCORES))],
                        ins=[cc_in[ch].ap()[:]], outs=[cc_out[ch].ap()[:]])

            # ---- BN scale/bias: s = gamma*rsqrt(var+eps), bb = beta - mu*s ----
            gsum = stat.tile([128, 2, 2], F32, tag="gsum")
            sbn = stat.tile([128, 2], F32, tag="sbn")
            bbn = stat.tile([128, 2], F32, tag="bbn")
            tmp = stat.tile([128, 2, 2], F32, tag="tmpbn")
            for ch in range(2):
                nc.sync.dma_start(out=gsum[:, ch, :], in_=cc_out[ch].ap()[:])
                mu, ex2 = tmp[:, ch, 0:1], tmp[:, ch, 1:2]
                nc.vector.tensor_scalar_mul(mu, gsum[:, ch, 0:1], 1.0 / NPIXG)
                nc.vector.tensor_scalar_mul(ex2, gsum[:, ch, 1:2], 1.0 / NPIXG)
                var = sbn[:, ch:ch + 1]
                nc.vector.tensor_scalar(var, mu, mu, None, OP.mult)
                nc.vector.tensor_sub(var, ex2, var)
                nc.scalar.activation(out=var, in_=var, func=AF.Sqrt, bias=eps128[:])
                nc.vector.reciprocal(out=var, in_=var)
                nc.vector.tensor_mul(var, var, bnpt[:, ch, 0:1])   # s = rstd * gamma
                nc.vector.tensor_mul(mu, mu, var)                  # mu*s
                nc.vector.tensor_sub(bbn[:, ch:ch + 1], bnpt[:, ch, 1:2], mu)

            # ---- phase 2: BN-apply+leaky (kc-major) -> conv1x1 -> LN stats ----
            with tc.tile_pool(name="yp", bufs=2) as yp, \
                 tc.tile_pool(name="blk", bufs=6) as blk:
                Y = [yp.tile([128, PIX], F32, tag="y", name=f"Y{i}") for i in range(2)]
                lnstat = stat.tile([128, 2, BL, 8, 6], F32, tag="lnstat")
                xbs = [[None, None] for _ in range(4)]
                for kc in range(2):              # chunk-0 applies can overlap AR1
                    for bi in range(4):
                        t = blk.tile([128, 2048], F32R, tag="xb", name=f"xb_{bi}_{kc}")
                        xbs[bi][kc] = t
                        nc.scalar.activation(
                            out=t[:], in_=X[kc][:, bi * 2048:(bi + 1) * 2048],
                            func=AF.Prelu, bias=bbn[:, kc:kc + 1], scale=sbn[:, kc:kc + 1],
                            alpha=ALPHA)
                for bi in range(4):              # blocks of 2048 pixels
                    for ch in range(2):
                        for sl in range(4):
                            acc = ps.tile([128, 512], F32, tag="ps",
                                          name=f"acy_{bi}_{ch}_{sl}")
                            for kc in range(2):
                                nc.tensor.matmul(
                                    acc, wvt[:, kc, ch * 128:(ch + 1) * 128],
                                    xbs[bi][kc][:, sl * 512:(sl + 1) * 512],
                                    start=(kc == 0), stop=(kc == 1))
                            seg = Y[ch][:, bi * 2048 + sl * 512: bi * 2048 + (sl + 1) * 512]
                            nc.scalar.activation(out=seg, in_=acc, func=AF.Identity,
                                                 bias=bnpt[:, ch, 2:3], scale=1.0)
                            nc.vector.bn_stats(out=lnstat[:, ch, bi // 2, (bi % 2) * 4 + sl, :],
                                               in_=seg)

                # ---- LN: combine per-channel stats across 256 channels ----
                rhsT = stat.tile([128, 2, 4], F32, tag="rhsT")   # (m_b0, e2_b0, m_b1, e2_b1)
                mvb = stat.tile([128, 2], F32, tag="mvb")
                for ch in range(2):
                    for b in range(BL):
                        nc.vector.bn_aggr(out=mvb[:], in_=lnstat[:, ch, b, :, :])
                        mean, var = mvb[:, 0:1], mvb[:, 1:2]
                        nc.vector.tensor_copy(rhsT[:, ch, 2 * b:2 * b + 1], mean)
                        nc.vector.tensor_scalar(rhsT[:, ch, 2 * b + 1:2 * b + 2],
                                                mean, mean, var, OP.mult, OP.add)
                ones = stat.tile([128, 1], F32, tag="ones")
                nc.vector.memset(ones[:], 1.0)
                with tc.tile_pool(name="pss", bufs=1, space="PSUM") as pss:
                    psum4 = pss.tile([1, 4], F32, tag="ps4")
                    for ch in range(2):
                        nc.tensor.matmul(psum4[:], ones[:], rhsT[:, ch, :],
                                         start=(ch == 0), stop=(ch == 1))
                    t4 = stat.tile([1, 2, 2], F32, tag="t4")     # [b, (m, e2)]
                    nc.scalar.activation(out=t4[:],
                                         in_=psum4[:].rearrange("p (b s) -> p b s", b=2),
                                         func=AF.Copy, scale=1.0 / C)
                    eps1 = stat.tile([1, 1], F32, tag="eps1")
                    nc.vector.memset(eps1[:], EPS)
                    m2 = stat.tile([1, 2], F32, tag="m2")
                    nc.vector.tensor_mul(m2[:], t4[:, :, 0], t4[:, :, 0])
                    varb = stat.tile([1, 2], F32, tag="varb")
                    nc.vector.tensor_sub(varb[:], t4[:, :, 1], m2[:])
                    nc.scalar.activation(out=varb[:], in_=varb[:], func=AF.Sqrt, bias=eps1[:])
                    nc.vector.reciprocal(out=varb[:], in_=varb[:])   # r_b
                    bsrc = stat.tile([1, 4], F32, tag="bsrc")        # (m0, m1, r0, r1)
                    nc.vector.tensor_copy(bsrc[:, 0:2], t4[:, :, 0])
                    nc.vector.tensor_copy(bsrc[:, 2:4], varb[:])
                    ones1 = stat.tile([1, 128], F32, tag="ones1")
                    nc.vector.memset(ones1[:], 1.0)
                    psbc = pss.tile([128, 4], F32, tag="psbc")
                    nc.tensor.matmul(psbc[:], ones1[:], bsrc[:], start=True, stop=True)
                    bc = stat.tile([128, 4], F32, tag="bc")          # (m0, m1, r0, r1) bcast
                    nc.scalar.activation(out=bc[:], in_=psbc[:], func=AF.Copy)
                lnbias = stat.tile([128, 2], F32, tag="lnbias")      # -m_b * r_b
                nc.vector.tensor_mul(lnbias[:], bc[:, 0:2], bc[:, 2:4])
                nc.vector.tensor_scalar_mul(lnbias[:], lnbias[:], -1.0)

                # ---- final: out = leaky((y - m_b) * r_b [* g + beta]) ----
                if fast_ln:
                    for ch in range(2):
                        outt = Xp.tile([128, PIX], F32, tag="X", name=f"out{ch}")
                        for b in range(BL):
                            nc.scalar.activation(
                                out=outt[:, b * 4096:(b + 1) * 4096],
                                in_=Y[ch][:, b * 4096:(b + 1) * 4096],
                                func=AF.Prelu, bias=lnbias[:, b:b + 1],
                                scale=bc[:, 2 + b:3 + b], alpha=ALPHA)
                            nc.sync.dma_start(
                                out=yout.ap()[ch * 128:(ch + 1) * 128,
                                              b * 4096:(b + 1) * 4096],
                                in_=outt[:, b * 4096:(b + 1) * 4096])
                else:
                    with tc.tile_pool(name="lnp", bufs=2) as lnp:
                        for ch in range(2):
                            gam = lnp.tile([128, H * W], F32, tag="gam", name=f"g{ch}")
                            bet = lnp.tile([128, H * W], F32, tag="bet", name=f"b{ch}")
                            nc.sync.dma_start(out=gam[:], in_=lng.ap()[ch * 128:(ch + 1) * 128, :])
                            nc.sync.dma_start(out=bet[:], in_=lnb.ap()[ch * 128:(ch + 1) * 128, :])
                            outt = Xp.tile([128, PIX], F32, tag="X", name=f"out{ch}")
                            for b in range(BL):
                                seg = outt[:, b * 4096:(b + 1) * 4096]
                                nc.scalar.activation(
                                    out=seg, in_=Y[ch][:, b * 4096:(b + 1) * 4096],
                                    func=AF.Identity, bias=lnbias[:, b:b + 1],
                                    scale=bc[:, 2 + b:3 + b])
                                nc.vector.tensor_mul(seg, seg, gam[:])
                                nc.vector.tensor_add(seg, seg, bet[:])
                                nc.scalar.activation(out=seg, in_=seg, func=AF.Prelu,
                                                     bias=0.0, scale=1.0, alpha=ALPHA)
                                nc.sync.dma_start(
                                    out=yout.ap()[ch * 128:(ch + 1) * 128,
                                                  b * 4096:(b + 1) * 4096],
                                    in_=seg)

    nc.compile()
    return nc


def kernel(**inputs):
    global LAST_RESULT
    x = np.ascontiguousarray(np.asarray(inputs["inputs"], dtype=np.float32))
    cbl_w = np.asarray(inputs["cbl_w"], dtype=np.float32)
    bn_gamma = np.asarray(inputs["bn_gamma"], dtype=np.float32)
    bn_beta = np.asarray(inputs["bn_beta"], dtype=np.float32)
    wv = np.asarray(inputs["wv"], dtype=np.float32).reshape(C, C)
    bv = np.asarray(inputs["bv"], dtype=np.float32)
    ln_gamma = np.asarray(inputs["ln_gamma"], dtype=np.float32)
    ln_beta = np.asarray(inputs["ln_beta"], dtype=np.float32)

    fast_ln = bool(np.all(ln_gamma == 1.0) and np.all(ln_beta == 0.0))

    # host-side repack (free for HW time): channel-major, pre-padded input
    xp = np.zeros((NCORES, CIN, BL, HP, WP), np.float32)
    xp[:, :, :, 1:H + 1, 1:W + 1] = (
        x.reshape(NCORES, BL, H, W, CIN).transpose(0, 4, 1, 2, 3))
    xin = np.ascontiguousarray(xp.reshape(NCORES, CIN, BL * HP * WP))
    cw = np.ascontiguousarray(cbl_w.transpose(2, 0, 1, 3).reshape(CIN, 9 * C))
    wv_eff = np.ascontiguousarray(wv + np.eye(C, dtype=np.float32))
    bnp = np.ascontiguousarray(np.stack([bn_gamma, bn_beta, bv], axis=1))

    if fast_ln not in _CACHE:
        _CACHE[fast_ln] = _build(fast_ln)
    nc = _CACHE[fast_ln]

    in_maps = []
    for i in range(NCORES):
        m = {"xin": xin[i], "cw": cw, "wv": wv_eff, "bnp": bnp}
        if not fast_ln:
            m["lng"] = np.ascontiguousarray(
                ln_gamma.transpose(2, 0, 1).reshape(C, H * W))
            m["lnb"] = np.ascontiguousarray(
                ln_beta.transpose(2, 0, 1).reshape(C, H * W))
        in_maps.append(m)

    res = run_bass_kernel_spmd(nc, in_maps, core_ids=list(range(NCORES)))
    LAST_RESULT = res

    out = np.empty((B, H, W, C), np.float32)
    for i in range(NCORES):
        yc = res.results[i]["yout"].reshape(C, BL, H, W)
        out[i * BL:(i + 1) * BL] = yc.transpose(1, 2, 3, 0)
    return out
